# revision 6
# baseline (speedup 1.0000x reference)
"""Trainium2 Bass kernel for the H2+ ion PINN loss (nn_NN_ion_52347061403910).

Math: psi = dec(R)*g(f1,f2) + f1 + f2 with f_i = exp(-r_i) and g the
symmetrized 2-16-16-1 MLP head.  The Laplacian needs (g, g1, g2) plus the
Hessian quadratic form  Q:Hg  with Q = [[f1^2, f1*f2*c],[f1*f2*c, f2^2]]
(c = cos angle between grad r1, grad r2).  Q factors as w w^T + u u^T with
w = (f1, f2*c), u = (0, f2*s), s^2 = 1-c^2, so

  Q:Hg = D^2_w g + (f2*s)^2 * g22

D^2_w g is one forward-over-forward tangent sweep (s1 = W1 v, s2 = W2(sig'*s1))
and g22 is the fixed-direction (0,1) second derivative via d2 = W2(sig'*wb).
Sigmoids are evaluated in tanh half-angle form (sig = (1+tanh(z/2))/2), which
makes sig' = (1-tau^2)/4 and sig'' = -tau(1-tau^2)/4 polynomial in tau, and
lets layer weights/biases fold into matmul weights and ACT scale/bias.
E(R), dec(R) are runtime-fitted Chebyshev polynomials (fused Horner STTs).

Layout: 8 cores pure data-parallel, 125000 pts/core, column-major padded to
128 x 977 so only the last column is partially valid (one [128,1] mask op).
Two column chunks [512 | 465].  Per chunk, pointwise geometry on [128,npc]
f32 tiles, then 32 bands of 4 rows x npc through the feature phase where the
16-wide MLP is packed 4 point-blocks x 2 branches x 16 = 128 partitions and
all GEMMs are single block-diagonal 128-wide fp16 matmuls (fp32 PSUM).  The
boundary term (psi at 2x8192 indices) is computed host-side in float64.
"""

import numpy as np
from contextlib import ExitStack

import concourse.bass as bass
from concourse import bacc
import concourse.tile as tile
import concourse.mybir as mybir
from concourse.bass_utils import run_bass_kernel_spmd

F32 = mybir.dt.float32
F16 = mybir.dt.float16
AT = mybir.ActivationFunctionType
OP = mybir.AluOpType

N_CORES = 8
N_TOTAL = 1_000_000
PER_CORE = N_TOTAL // N_CORES   # 125000
NROWS = 128
NF = 977                        # columns; 128*977 = 125056 >= 125000
PADDED = NROWS * NF
CHUNKS = (512, 465)
NVALID_LASTCOL = PER_CORE - (NF - 1) * NROWS  # 72 valid rows in col 976
DEG_E = 12
DEG_D = 12
CHEB_COLS = 3 + (DEG_E + 1) + (DEG_D + 1)


def _sigmoid(x):
    return 1.0 / (1.0 + np.exp(-x))


def _cheb_fit(f, lo, hi, deg):
    k = np.arange(deg + 1)
    tn = np.cos((2 * k + 1) * np.pi / (2 * (deg + 1)))
    y = f(0.5 * (tn + 1) * (hi - lo) + lo)
    c = np.polynomial.chebyshev.chebfit(tn, y, deg)
    pc = np.polynomial.chebyshev.cheb2poly(c)   # power basis in t = a*R+b
    tg = np.linspace(-1, 1, 4097)
    rg = 0.5 * (tg + 1) * (hi - lo) + lo
    err = np.abs(np.polynomial.polynomial.polyval(tg, pc) - f(rg)).max()
    return pc, err


# fp16 matmul weights; fp32 biases/scalars
W16 = ("WA", "WS", "W2bd", "WD2bd", "WCneg",
       "HG", "HVT2", "HVT", "HR1", "HR2", "HR22")
WEIGHT_SHAPES = dict(WA=(12, 128), WS=(12, 128), W2bd=(128, 128),
                     WD2bd=(128, 128), WCneg=(128, 128),
                     HG=(128, 20), HVT2=(128, 20), HVT=(128, 20),
                     HR1=(128, 20), HR2=(128, 20), HR22=(128, 20),
                     BT1=(128, 1), BT2=(128, 1), UB0=(128, 1))


def build_consts(params):
    """Host-side folded weight tensors (lhsT layout [K, M])."""
    p = {k: np.asarray(v, np.float64) for k, v in params.items()}
    W1 = p["W_H1"]            # [16,2]
    b1 = p["b_H1"]
    W2 = p["W_H2"]            # [16,16]
    b2 = p["b_H2"]
    Wo = p["W_out"][0]        # [16]
    w0, w1 = W1[:, 0], W1[:, 1]

    def wab(br):
        return (w0, w1) if br == 0 else (w1, w0)

    WA = np.zeros((12, 128))    # rhs rows: F1 x4, F2 x4, F2c x4
    WS = np.zeros((12, 128))    # pS = -s1/4, s1 = wa*F1 + wb*F2c
    for pb in range(4):
        for br in range(2):
            wa, wb = wab(br)
            cols = slice(32 * pb + 16 * br, 32 * pb + 16 * br + 16)
            WA[pb, cols] = wa
            WA[4 + pb, cols] = wb
            WS[pb, cols] = -wa / 4
            WS[8 + pb, cols] = -wb / 4

    W2bd = np.zeros((128, 128))   # z2 preact / tangent: out = W2 @ rhs
    WD2bd = np.zeros((128, 128))  # d2 = 0.25*W2 (wb * rhs)
    WCneg = np.zeros((128, 128))  # pC = -0.25*W2^T (Wo * rhs)
    for pb in range(4):
        for br in range(2):
            o = 32 * pb + 16 * br
            wa, wb = wab(br)
            # out[o+i] = sum_j lhsT[o+j, o+i] * rhs[o+j]
            W2bd[o:o + 16, o:o + 16] = W2.T
            WD2bd[o:o + 16, o:o + 16] = 0.25 * (W2 * wb[None, :]).T
            WCneg[o:o + 16, o:o + 16] = -0.25 * (Wo[:, None] * W2)

    # heads: rows of pH = 4*h + pb, h in [G, g1, g2, hw, g22]
    HG = np.zeros((128, 20))
    HVT2 = np.zeros((128, 20))
    HVT = np.zeros((128, 20))
    HR1 = np.zeros((128, 20))
    HR2 = np.zeros((128, 20))
    HR22 = np.zeros((128, 20))
    for pb in range(4):
        for br in range(2):
            r = slice(32 * pb + 16 * br, 32 * pb + 16 * br + 16)
            wa, wb = wab(br)
            HG[r, 0 + pb] = 0.5 * Wo
            HVT2[r, 4 + pb] = 0.25 * wa
            HVT2[r, 8 + pb] = 0.25 * wb
            HVT[r, 16 + pb] = -0.25 * wb * wb
            HR1[r, 12 + pb] = 0.25 * Wo
            HR2[r, 12 + pb] = -4.0
            HR22[r, 16 + pb] = 0.25 * Wo

    BT1 = np.tile(b1 / 2, 8)[:, None]
    BT2 = np.tile((b2 + 0.5 * W2.sum(1)) / 2, 8)[:, None]
    UB0 = np.tile(np.tile(0.25 * (Wo @ W2), 2), 4)[:, None]  # 0.25*W2^T Wo

    consts = dict(WA=WA, WS=WS, W2bd=W2bd, WD2bd=WD2bd, WCneg=WCneg,
                  HG=HG, HVT2=HVT2, HVT=HVT, HR1=HR1, HR2=HR2, HR22=HR22,
                  BT1=BT1, BT2=BT2, UB0=UB0)
    return {k: np.ascontiguousarray(v, np.float16 if k in W16 else np.float32)
            for k, v in consts.items()}


def build_cheb(params, R):
    """[128, CHEB_COLS]: cols [alpha, beta, c0, cE..., cD...]."""
    p = {k: np.asarray(v, np.float64) for k, v in params.items()}

    def E_of(r):
        e = _sigmoid(np.outer(r, p["W_E1"][:, 0]) + p["b_E1"])
        e = _sigmoid(e @ p["W_E2"].T + p["b_E2"])
        return e @ p["W_Eout"][0] + p["b_Eout"][0]

    def D_of(r):
        fd = _sigmoid(np.outer(r, p["W_DL"][:, 0]) + p["b_DL"])
        return fd @ p["W_D"][0] + p["b_D"][0]

    lo = float(np.min(R)) - 1e-6
    hi = float(np.max(R)) + 1e-6
    alpha = 2.0 / (hi - lo)
    beta = -(hi + lo) / (hi - lo)
    cE, eE = _cheb_fit(E_of, lo, hi, DEG_E)
    cD, eD = _cheb_fit(D_of, lo, hi, DEG_D)
    assert eE < 1e-3 and eD < 1e-3, (eE, eD)
    c0 = float(p["b_out"][0] + p["W_out"][0].sum())
    row = np.concatenate([[alpha, beta, c0], cE, cD])
    assert row.shape[0] == CHEB_COLS
    return np.ascontiguousarray(np.tile(row[None, :], (128, 1)), np.float32)


def build_bass(bench_repeat=1):
    nc = bacc.Bacc("TRN2", target_bir_lowering=False, debug=False)

    X = nc.dram_tensor("X", [NROWS, NF], F32, kind="ExternalInput")
    Y = nc.dram_tensor("Y", [NROWS, NF], F32, kind="ExternalInput")
    Z = nc.dram_tensor("Z", [NROWS, NF], F32, kind="ExternalInput")
    RT = nc.dram_tensor("RT", [NROWS, NF], F32, kind="ExternalInput")
    CHEB = nc.dram_tensor("CHEB", [NROWS, CHEB_COLS], F32, kind="ExternalInput")
    MCOL = nc.dram_tensor("MCOL", [NROWS, 1], F32, kind="ExternalInput")
    Wd = {nm: nc.dram_tensor(nm, list(shp), F16 if nm in W16 else F32,
                             kind="ExternalInput")
          for nm, shp in WEIGHT_SHAPES.items()}
    ACC_D = nc.dram_tensor("ACC", [NROWS, 1], F32, kind="ExternalOutput")

    v = nc.vector
    a = nc.scalar
    g = nc.gpsimd
    te = nc.tensor
    dma = nc.sync

    with tile.TileContext(nc) as tc, ExitStack() as ctx:
        cpool = ctx.enter_context(tc.tile_pool(name="consts", bufs=1))
        pw = ctx.enter_context(tc.tile_pool(name="pw", bufs=1))
        ft = ctx.enter_context(tc.tile_pool(name="ft", bufs=2))
        psA = ctx.enter_context(tc.tile_pool(name="psA", bufs=1, space="PSUM"))
        psS = ctx.enter_context(tc.tile_pool(name="psS", bufs=1, space="PSUM"))
        psB = ctx.enter_context(tc.tile_pool(name="psB", bufs=1, space="PSUM"))
        psT = ctx.enter_context(tc.tile_pool(name="psT", bufs=1, space="PSUM"))
        psD = ctx.enter_context(tc.tile_pool(name="psD", bufs=1, space="PSUM"))
        psC = ctx.enter_context(tc.tile_pool(name="psC", bufs=1, space="PSUM"))
        psH = ctx.enter_context(tc.tile_pool(name="psH", bufs=2, space="PSUM"))

        W = {}
        for nm in Wd:
            W[nm] = cpool.tile(list(WEIGHT_SHAPES[nm]),
                               F16 if nm in W16 else F32,
                               name=f"w_{nm}", tag=f"w_{nm}")
            dma.dma_start(W[nm][:], Wd[nm][:])
        CH = cpool.tile([NROWS, CHEB_COLS], F32, name="cheb", tag="cheb")
        dma.dma_start(CH[:], CHEB[:])
        MC = cpool.tile([NROWS, 1], F32, name="mcol", tag="mcol")
        dma.dma_start(MC[:], MCOL[:])

        def chb(i):
            return CH[:, i:i + 1]

        def body():
            acc_parts = []
            col0 = 0
            for c, npc in enumerate(CHUNKS):
                cs = slice(col0, col0 + npc)
                col0 += npc

                def pwt(tag, dt=F32):
                    return pw.tile([NROWS, npc], dt, name=tag, tag=tag)

                Xt, Yt, Zt, Rt = (pwt(t) for t in ("Xt", "Yt", "Zt", "Rt"))
                dma.dma_start(Xt[:], X[:, cs])
                dma.dma_start(Yt[:], Y[:, cs])
                dma.dma_start(Zt[:], Z[:, cs])
                dma.dma_start(Rt[:], RT[:, cs])

                # ---- PW geometry
                s_a, s_b, s_c = pwt("s_a"), pwt("s_b"), pwt("s_c")
                D1t, D2t, YZ2 = pwt("D1t"), pwt("D2t"), pwt("YZ2")
                g.tensor_sub(D1t[:], Xt[:], Rt[:])
                g.tensor_add(D2t[:], Xt[:], Rt[:])
                a.square(s_a[:], Yt[:])
                a.square(s_b[:], Zt[:])
                g.tensor_add(YZ2[:], s_a[:], s_b[:])
                R1t, R2t = pwt("R1t"), pwt("R2t")
                a.square(s_a[:], D1t[:])
                g.tensor_add(s_a[:], s_a[:], YZ2[:])
                a.sqrt(R1t[:], s_a[:])
                a.square(s_b[:], D2t[:])
                g.tensor_add(s_b[:], s_b[:], YZ2[:])
                a.sqrt(R2t[:], s_b[:])
                Q1t, Q2t = pwt("Q1t"), pwt("Q2t")
                v.reciprocal_approx_fast(out=Q1t[:], in_=R1t[:])
                v.reciprocal_approx_fast(out=Q2t[:], in_=R2t[:])
                F1t, F2t = pwt("F1t"), pwt("F2t")
                a.activation(F1t[:], R1t[:], AT.Exp, scale=-1.0)
                a.activation(F2t[:], R2t[:], AT.Exp, scale=-1.0)
                # c12 = (d1*d2 + yz2) * q1 * q2 ; F2c = f2*c12
                F2Ct, W2SSt = pwt("F2Ct"), pwt("W2SSt")
                v.tensor_mul(s_a[:], D1t[:], D2t[:])
                g.tensor_add(s_a[:], s_a[:], YZ2[:])
                v.tensor_mul(s_b[:], Q1t[:], Q2t[:])
                v.tensor_mul(s_c[:], s_a[:], s_b[:])
                g.tensor_mul(F2Ct[:], F2t[:], s_c[:])
                # fp16 copies feeding the band rhs DMAs
                F1h = pwt("F1h", F16)
                F2h = pwt("F2h", F16)
                F2Ch = pwt("F2Ch", F16)
                v.tensor_copy(F1h[:], F1t[:])
                v.tensor_copy(F2h[:], F2t[:])
                v.tensor_copy(F2Ch[:], F2Ct[:])
                # W2SS = f2^2 - F2c^2 = f2^2 * (1-c^2)
                a.square(s_a[:], F2t[:])
                a.square(s_b[:], F2Ct[:])
                v.tensor_sub(W2SSt[:], s_a[:], s_b[:])
                # lap f_i:  S_iL = f_i * (1 - 2 q_i)
                S1Lt, S2Lt = pwt("S1Lt"), pwt("S2Lt")
                v.tensor_scalar(s_a[:], Q1t[:], -2.0, 1.0, OP.mult, OP.add)
                g.tensor_mul(S1Lt[:], s_a[:], F1t[:])
                v.tensor_scalar(s_b[:], Q2t[:], -2.0, 1.0, OP.mult, OP.add)
                g.tensor_mul(S2Lt[:], s_b[:], F2t[:])
                # pot + E
                POTEt = pwt("POTEt")
                g.tensor_add(POTEt[:], Q1t[:], Q2t[:])

                # ---- Chebyshev E(R), dec(R)
                RN = pwt("RN")
                v.tensor_scalar(RN[:], Rt[:], chb(0), chb(1), OP.mult, OP.add)
                EEt, DECt = pwt("EEt"), pwt("DECt")

                def horner(eng, out, base, deg):
                    eng.tensor_scalar_mul(out[:], RN[:], chb(base + deg))
                    for k in range(deg - 1, 0, -1):
                        eng.scalar_tensor_tensor(out[:], out[:], chb(base + k),
                                                 RN[:], OP.add, OP.mult)
                    eng.tensor_scalar_add(out[:], out[:], chb(base))

                horner(v, EEt, 3, DEG_E)
                horner(v, DECt, 3 + DEG_E + 1, DEG_D)
                v.tensor_add(POTEt[:], POTEt[:], EEt[:])

                Gh, G1h, G2h = pwt("Gh"), pwt("G1h"), pwt("G2h")
                HWh, G22h = pwt("HWh"), pwt("G22h")

                # ---- feature phase: 32 bands of 4 rows
                for b in range(NROWS // 4):
                    rows = slice(4 * b, 4 * b + 4)
                    RH = ft.tile([12, npc], F16, name="RH", tag="RH")
                    dma.dma_start(RH[0:4, :], F1h[rows, :])
                    dma.dma_start(RH[4:8, :], F2h[rows, :])
                    dma.dma_start(RH[8:12, :], F2Ch[rows, :])

                    def ftt(tag, dt=F16):
                        return ft.tile([128, npc], dt, name=tag, tag=tag)

                    pA = psA.tile([128, npc], F32, name="pA", tag="pA")
                    te.matmul(pA[:], W["WA"][:], RH[:], start=True, stop=True)
                    pS = psS.tile([128, npc], F32, name="pS", tag="pS")
                    te.matmul(pS[:], W["WS"][:], RH[:], start=True, stop=True)

                    TAU1 = ftt("TAU1")
                    a.activation(TAU1[:], pA[:], AT.Tanh,
                                 bias=W["BT1"][:, 0:1], scale=0.5)
                    TSQ1 = ftt("TSQ1")
                    g.tensor_mul(TSQ1[:], TAU1[:], TAU1[:])
                    SP1 = ftt("SP1")  # 1 - tau1^2 = 4 sig'(z1)
                    v.tensor_scalar(SP1[:], TSQ1[:], 1.0, -1.0,
                                    OP.subtract, OP.mult)
                    T1 = ftt("T1")    # (tsq1-1)*pS = sig'(z1)*s1
                    v.scalar_tensor_tensor(T1[:], TSQ1[:], 1.0, pS[:],
                                           OP.subtract, OP.mult)
                    SQS1 = ftt("SQS1")  # s1^2/16
                    a.square(SQS1[:], pS[:])

                    pB = psB.tile([128, npc], F32, name="pB", tag="pB")
                    te.matmul(pB[:], W["W2bd"][:], TAU1[:],
                              start=True, stop=True)
                    pT = psT.tile([128, npc], F32, name="pT", tag="pT")
                    te.matmul(pT[:], W["W2bd"][:], T1[:], start=True, stop=True)
                    pD2 = psD.tile([128, npc], F32, name="pD2", tag="pD2")
                    te.matmul(pD2[:], W["WD2bd"][:], SP1[:],
                              start=True, stop=True)

                    TAU2 = ftt("TAU2")
                    a.activation(TAU2[:], pB[:], AT.Tanh,
                                 bias=W["BT2"][:, 0:1], scale=0.25)
                    TSQ2 = ftt("TSQ2")
                    v.tensor_mul(TSQ2[:], TAU2[:], TAU2[:])
                    UTpp = ftt("UTpp")  # (tsq2-1)*tau2 = 4 sig''(z2)
                    v.scalar_tensor_tensor(UTpp[:], TSQ2[:], 1.0, TAU2[:],
                                           OP.subtract, OP.mult)
                    pC = psC.tile([128, npc], F32, name="pC", tag="pC")
                    te.matmul(pC[:], W["WCneg"][:], TSQ2[:],
                              start=True, stop=True)
                    SQS2 = ftt("SQS2")  # s2^2
                    a.square(SQS2[:], pT[:])
                    DSQ2 = ftt("DSQ2")  # d2h^2
                    a.square(DSQ2[:], pD2[:])

                    VT2 = ftt("VT2")  # (pC + ub0) * SP1 = ubar * 4 sig'(z1)
                    v.scalar_tensor_tensor(VT2[:], pC[:], W["UB0"][:, 0:1],
                                           SP1[:], OP.add, OP.mult)
                    VT = ftt("VT")
                    v.tensor_mul(VT[:], VT2[:], TAU1[:])
                    R1 = ftt("R1")
                    v.tensor_mul(R1[:], UTpp[:], SQS2[:])
                    R2 = ftt("R2")
                    v.tensor_mul(R2[:], VT[:], SQS1[:])
                    R22 = ftt("R22")
                    g.tensor_mul(R22[:], UTpp[:], DSQ2[:])

                    pH = psH.tile([20, npc], F32, name="pH", tag="pH")
                    te.matmul(pH[:], W["HG"][:], TAU2[:],
                              start=True, stop=False)
                    te.matmul(pH[:], W["HVT2"][:], VT2[:],
                              start=False, stop=False)
                    te.matmul(pH[:], W["HVT"][:], VT[:],
                              start=False, stop=False)
                    te.matmul(pH[:], W["HR1"][:], R1[:],
                              start=False, stop=False)
                    te.matmul(pH[:], W["HR2"][:], R2[:],
                              start=False, stop=False)
                    te.matmul(pH[:], W["HR22"][:], R22[:],
                              start=False, stop=True)
                    HST = ft.tile([20, npc], F32, name="HST", tag="HST")
                    v.tensor_copy(HST[:], pH[:])
                    dma.dma_start(Gh[rows, :], HST[0:4, :])
                    dma.dma_start(G1h[rows, :], HST[4:8, :])
                    dma.dma_start(G2h[rows, :], HST[8:12, :])
                    dma.dma_start(HWh[rows, :], HST[12:16, :])
                    dma.dma_start(G22h[rows, :], HST[16:20, :])

                # ---- assembly
                PSIt, LAPt = pwt("PSIt"), pwt("LAPt")
                v.tensor_scalar_add(s_a[:], Gh[:], chb(2))
                g.tensor_mul(PSIt[:], s_a[:], DECt[:])
                v.tensor_add(PSIt[:], PSIt[:], F1t[:])
                v.tensor_add(PSIt[:], PSIt[:], F2t[:])
                v.tensor_mul(s_a[:], W2SSt[:], G22h[:])
                g.tensor_add(s_a[:], s_a[:], HWh[:])
                v.tensor_mul(s_b[:], G1h[:], S1Lt[:])
                g.tensor_add(s_a[:], s_a[:], s_b[:])
                v.tensor_mul(s_c[:], G2h[:], S2Lt[:])
                g.tensor_add(s_a[:], s_a[:], s_c[:])
                v.tensor_mul(LAPt[:], DECt[:], s_a[:])
                g.tensor_add(LAPt[:], LAPt[:], S1Lt[:])
                v.tensor_add(LAPt[:], LAPt[:], S2Lt[:])
                REST = pwt("REST")
                v.tensor_mul(s_a[:], POTEt[:], PSIt[:])
                v.scalar_tensor_tensor(REST[:], LAPt[:], -0.5, s_a[:],
                                       OP.mult, OP.subtract)
                if c == len(CHUNKS) - 1:
                    v.tensor_mul(REST[:, npc - 1:npc],
                                 REST[:, npc - 1:npc], MC[:, 0:1])
                acc_c = cpool.tile([NROWS, 1], F32, name=f"acc{c}",
                                   tag=f"acc{c}")
                a.activation(s_a[:], REST[:], AT.Square, accum_out=acc_c[:])
                acc_parts.append(acc_c)

            tot = cpool.tile([NROWS, 1], F32, name="acctot", tag="acctot")
            v.tensor_add(tot[:], acc_parts[0][:], acc_parts[1][:])
            dma.dma_start(ACC_D[:], tot[:])

        if bench_repeat > 1:
            with tc.For_i(0, bench_repeat, 1):
                body()
        else:
            body()

    nc.compile()
    return nc


def make_in_maps(inputs):
    params = {k: v for k, v in inputs.items() if k not in
              ("x", "y", "z", "R", "bIndex1", "bIndex2")}
    consts = build_consts(params)
    cheb = build_cheb(params, np.asarray(inputs["R"], np.float32))
    mcol = (np.arange(NROWS) < NVALID_LASTCOL).astype(np.float32)[:, None]

    in_maps = []
    for core in range(N_CORES):
        sl = slice(core * PER_CORE, (core + 1) * PER_CORE)

        def shard(arr, fill):
            s = np.asarray(arr, np.float32)[sl, 0]
            buf = np.full((NF, NROWS), fill, np.float32)
            buf.reshape(-1)[:PER_CORE] = s
            return np.ascontiguousarray(buf.T)   # [128, NF], col-major points

        m = dict(consts)
        m["X"] = shard(inputs["x"], 0.5)
        m["Y"] = shard(inputs["y"], 0.5)
        m["Z"] = shard(inputs["z"], 0.5)
        m["RT"] = shard(inputs["R"], 1.0)
        m["CHEB"] = cheb
        m["MCOL"] = mcol
        in_maps.append(m)
    return in_maps


def host_boundary(inputs):
    """Lbc = mean(psi[b1]^2) + mean(psi[b2]^2), float64 host computation."""
    p = {k: np.asarray(v, np.float64) for k, v in inputs.items()
         if k.startswith(("W_", "b_"))}
    idx = np.concatenate([np.asarray(inputs["bIndex1"]).astype(np.int64),
                          np.asarray(inputs["bIndex2"]).astype(np.int64)])
    x = np.asarray(inputs["x"], np.float64)[idx, 0]
    y = np.asarray(inputs["y"], np.float64)[idx, 0]
    z = np.asarray(inputs["z"], np.float64)[idx, 0]
    R = np.asarray(inputs["R"], np.float64)[idx, 0]
    r1 = np.sqrt((x - R) ** 2 + y ** 2 + z ** 2)
    r2 = np.sqrt((x + R) ** 2 + y ** 2 + z ** 2)
    f1, f2 = np.exp(-r1), np.exp(-r2)
    W1, b1 = p["W_H1"], p["b_H1"]
    W2, b2 = p["W_H2"], p["b_H2"]
    B = 0.0
    for (aa, bb) in ((f1, f2), (f2, f1)):
        h = _sigmoid(np.outer(aa, W1[:, 0]) + np.outer(bb, W1[:, 1]) + b1)
        B = B + _sigmoid(h @ W2.T + b2)
    fd = _sigmoid(np.outer(R, p["W_DL"][:, 0]) + p["b_DL"])
    dec = fd @ p["W_D"][0] + p["b_D"][0]
    psi = ((B @ p["W_out"][0]) + p["b_out"][0]) * dec + f1 + f2
    n = idx.shape[0] // 2
    return float((psi[:n] ** 2).mean() + (psi[n:] ** 2).mean())


_NC_CACHE = {}


def kernel(**inputs):
    if "nc" not in _NC_CACHE:
        _NC_CACHE["nc"] = build_bass()
    nc = _NC_CACHE["nc"]

    in_maps = make_in_maps(inputs)
    results = run_bass_kernel_spmd(nc, in_maps, core_ids=list(range(N_CORES)))
    outs = results.results

    res2 = float(sum(np.asarray(outs[c]["ACC"], np.float64).sum()
                     for c in range(N_CORES)))
    loss = res2 / N_TOTAL + host_boundary(inputs)
    return np.float32(loss)
